# revision 1
# baseline (speedup 1.0000x reference)
"""MiniBatchDiscrimination kernel, v4: symmetric-pair sharding +
column-tiled paired PE reduction.

Math per core (row block of 64 i's x FD=320 j columns spanning 5 blocks):
  Mt[(o,k), j] = M^T in bf16 (16 partition-tiles), computed on PE.
  For each i:
    relu tiles (DVE, 4x bf16 tensor_scalar):  R_t = max(Mt_t - Mt_t[:,i], 0)
    abs tiles  (ACT offload, 2 tiles):        A_t = |Mt_t - Mt_t[:,i]|
    D[o,j] = sum_A |d| + 2*sum_R relu(d) - (S_j - S_i),  S = sum_k Mt (relu
    tiles only), so D = fold(psumA + psumB) with S_i applied as the Exp bias.
  The 16+1 reduction matmuls are issued as 8 column-tiled PAIRS: tile 2p ->
  PSUM partitions 0:64 (col group 0-1), tile 2p+1 -> partitions 64:128 (col
  group 2-3, tile_position=(0,64)).  The two streams run concurrently on
  disjoint array column groups, nearly halving PE time.  A DVE tensor_add
  folds the halves into a fresh bf16 tile; Exp(scale=-1, bias=-S_i,
  accum_out=rowsum) reads the fold.  The fold is software-pipelined one
  iteration behind the matmuls so its PE wait never stalls the DVE - and it
  doubles as DVE's PE-clock refresh, so slot-reuse waits are pre-observed
  (the walrus here encodes at most ONE sync wait per instruction).
  Column-sum partials (for the partner blocks, by symmetry) are reduced on
  PE from the packed exp tiles once per 8 rows and accumulated in fp32.
"""

import numpy as np
import ml_dtypes
from contextlib import ExitStack

BATCH, IN_FEAT, OUT_FEAT, KERNEL_DIM = 512, 512, 64, 32
N_CORES = 8
ROWB = BATCH // N_CORES          # 64 rows of i per core
OK = OUT_FEAT * KERNEL_DIM       # 2048 flattened (o,k)
NT = OK // 128                   # 16 partition-tiles of (o,k)
NBLK = 5                         # column blocks per core
FD = NBLK * 64                   # 320
POISON = 1.0e4

CHUNK = 16                       # i's per colsum PSUM chunk
SELW = OUT_FEAT
ACT_TILES = (5, 7, 11)           # elementwise tiles computed on ACT as Abs
ADV_BUFS = 56                    # 4 iterations of DVE elementwise tiles

_cache = {}


def _build_nc(split_waits=True):
    import concourse.bass as bass
    import concourse.mybir as mybir
    import concourse.tile as tile

    dt = mybir.dt
    AF = mybir.ActivationFunctionType
    OP = mybir.AluOpType

    nc = bass.Bass("TRN2", target_bir_lowering=False, debug=False,
                   num_devices=N_CORES)

    xT_d = nc.dram_tensor("xT", [IN_FEAT, FD], dt.bfloat16, kind="ExternalInput")
    T_d = nc.dram_tensor("Tm", [IN_FEAT, OK], dt.bfloat16, kind="ExternalInput")
    sel_d = nc.dram_tensor("sel", [128, NT * SELW], dt.bfloat16,
                           kind="ExternalInput")
    sel2_d = nc.dram_tensor("sel2", [128, OUT_FEAT], dt.bfloat16,
                            kind="ExternalInput")
    selS_d = nc.dram_tensor("selS", [128, NT * SELW], dt.bfloat16,
                            kind="ExternalInput")
    dneg_d = nc.dram_tensor("dneg", [OUT_FEAT, OUT_FEAT], dt.bfloat16,
                            kind="ExternalInput")
    rows_d = nc.dram_tensor("rowS", [OUT_FEAT, ROWB], dt.float32,
                            kind="ExternalOutput")
    acc_d = nc.dram_tensor("accS", [OUT_FEAT, FD], dt.float32,
                           kind="ExternalOutput")

    with tile.TileContext(nc) as tc, ExitStack() as ctx:
        const = ctx.enter_context(tc.tile_pool(name="const", bufs=1))
        mtp = ctx.enter_context(tc.tile_pool(name="mt", bufs=NT))
        psA = ctx.enter_context(
            tc.tile_pool(name="psA", bufs=1, space=bass.MemorySpace.PSUM))
        psDA = ctx.enter_context(
            tc.tile_pool(name="psDA", bufs=3, space=bass.MemorySpace.PSUM))
        psDB = ctx.enter_context(
            tc.tile_pool(name="psDB", bufs=2, space=bass.MemorySpace.PSUM))
        psC = ctx.enter_context(
            tc.tile_pool(name="psC", bufs=1, space=bass.MemorySpace.PSUM))
        workV = ctx.enter_context(tc.tile_pool(name="workV", bufs=ADV_BUFS))
        # ACT-written tiles (Abs elementwise + Exp outputs) share one pool:
        # the WAW chain keeps ACT's scheduled order near program order
        ep = ctx.enter_context(tc.tile_pool(name="e", bufs=64))
        # fold outputs are fresh (never reused) so the fold and the Exp that
        # reads it each carry exactly one wait
        foldp = ctx.enter_context(tc.tile_pool(name="fold", bufs=ROWB))

        Tsb = []
        for kc in range(4):
            t_ = const.tile([128, OK], dt.bfloat16, tag=f"T{kc}")
            nc.sync.dma_start(t_[:], T_d[kc * 128:(kc + 1) * 128, :])
            Tsb.append(t_)
        xTsb = []
        for kc in range(4):
            t_ = const.tile([128, FD], dt.bfloat16, tag=f"x{kc}")
            nc.sync.dma_start(t_[:], xT_d[kc * 128:(kc + 1) * 128, :])
            xTsb.append(t_)
        sel = const.tile([128, NT * SELW], dt.bfloat16, tag="sel")
        nc.sync.dma_start(sel[:], sel_d[:])
        sel2 = const.tile([128, OUT_FEAT], dt.bfloat16, tag="sel2")
        nc.sync.dma_start(sel2[:], sel2_d[:])
        selS = const.tile([128, NT * SELW], dt.bfloat16, tag="selS")
        nc.sync.dma_start(selS[:], selS_d[:])
        dneg = const.tile([OUT_FEAT, OUT_FEAT], dt.bfloat16, tag="dneg")
        nc.sync.dma_start(dneg[:], dneg_d[:])
        mcol = const.tile([128, NT * ROWB], dt.float32, tag="mcol")
        rowS = const.tile([OUT_FEAT, ROWB], dt.float32, tag="rowS")
        accS = const.tile([OUT_FEAT, FD], dt.float32, tag="accS")
        nc.vector.memset(accS[:], 0.0)

        # Mt tiles: Mt[(o,k), j], tile t holds o in [4t, 4t+4), all k
        r_tiles = [t for t in range(NT) if t not in ACT_TILES]
        psS = psA.tile([OUT_FEAT, FD], dt.float32, tag="psS")
        mts = []
        for t in range(NT):
            ps = psA.tile([128, FD], dt.float32)
            for kc in range(4):
                nc.tensor.matmul(ps[:],
                                 Tsb[kc][:, t * 128:(t + 1) * 128],
                                 xTsb[kc][:],
                                 start=(kc == 0), stop=(kc == 3))
            mt_t = mtp.tile([128, FD], dt.bfloat16, tag="mt")
            nc.vector.tensor_copy(mt_t[:], ps[:])
            # scalar columns: the *rounded* bf16 values recast to fp32 so the
            # diagonal difference is exactly zero
            nc.vector.tensor_copy(mcol[:, t * ROWB:(t + 1) * ROWB],
                                  mt_t[:, 0:ROWB])
            mts.append(mt_t)

        # S[o, j] = sum_k Mt[(o,k), j] over the relu tiles only; kept in
        # bf16 so the Exp bias cancels the matmul term exactly on the
        # diagonal: D_ii = 2*0 + S_i - S_i = 0.
        for m, t in enumerate(r_tiles):
            nc.tensor.matmul(psS[:], selS[:, t * SELW:(t + 1) * SELW],
                             mts[t][:], start=(m == 0),
                             stop=(m == len(r_tiles) - 1))
        S_bf = const.tile([OUT_FEAT, FD], dt.bfloat16, tag="S_bf")
        nc.vector.tensor_copy(S_bf[:], psS[:])
        Sneg = const.tile([OUT_FEAT, ROWB], dt.float32, tag="Sneg")
        nc.vector.tensor_scalar(Sneg[:], S_bf[:, 0:ROWB], -1.0, None,
                                op0=OP.mult)
        # warm up ACT's observed DVE clock so the first ACT op (reading
        # DVE-written tiles) does not need a second sync wait
        warmA = const.tile([1, 1], dt.float32, tag="warmA")
        nc.scalar.copy(warmA[:], Sneg[0:1, 0:1])

        e_tiles_of = {}          # chunk -> list of packed e tiles
        pending = None           # (psd2, i) awaiting fold+exp

        def fold_exp(pair, i):
            psda, psdb = pair
            # move the B half to SBUF (bf16) and fold it into the A bank on
            # PE via an identity matmul (sel2's top half is I64); the copy
            # doubles as DVE's PE-clock refresh
            b_sb = foldp.tile([OUT_FEAT, FD], dt.bfloat16, tag="fold",
                              name=f"bsb_{i}")
            nc.vector.tensor_copy(b_sb[:], psdb[OUT_FEAT:128, :])
            nc.tensor.matmul(psda[:], sel2[0:OUT_FEAT, :],
                             b_sb[:], start=False, stop=True)
            ch = i // CHUNK
            if i % 2 == 0:
                e_t = ep.tile([128, FD], dt.bfloat16, tag="e",
                              name=f"e_{i}")
                e_tiles_of.setdefault(ch, []).append(e_t)
            half = e_tiles_of[ch][-1][(i % 2) * OUT_FEAT:
                                      (i % 2 + 1) * OUT_FEAT, :]
            nc.scalar.activation(half, psda[:], AF.Exp,
                                 scale=-1.0, bias=Sneg[:, i:i + 1],
                                 accum_out=rowS[:, i:i + 1])
            if i % CHUNK == CHUNK - 1:
                # column-sum partials for this chunk
                psc = psC.tile([OUT_FEAT, FD], dt.float32)
                ets = e_tiles_of[ch]
                for m, e_t in enumerate(ets):
                    nc.tensor.matmul(psc[:], sel2[:], e_t[:],
                                     start=(m == 0),
                                     stop=(m == len(ets) - 1))
                nc.vector.tensor_add(accS[:], accS[:], psc[:])

        for i in range(ROWB):
            psda = psDA.tile([OUT_FEAT, FD], dt.float32, tag="psda",
                             name=f"psda_{i}")
            psdb = psDB.tile([128, FD], dt.float32, tag="psdb",
                             name=f"psdb_{i}")
            ads = {}
            for t in range(NT):
                sc = mcol[:, t * ROWB + i: t * ROWB + i + 1]
                if t in ACT_TILES:
                    ad_t = ep.tile([128, FD], dt.bfloat16, tag="e",
                                   name=f"adA_{i}_{t}")
                    nc.scalar.activation(ad_t[:], mts[t][:], AF.Abs,
                                         bias=sc, scale=-1.0)
                else:
                    ad_t = workV.tile([128, FD], dt.bfloat16, tag="adV",
                                      name=f"ad_{i}_{t}")
                    nc.vector.tensor_scalar(ad_t[:], mts[t][:], sc, 0.0,
                                            op0=OP.subtract, op1=OP.max)
                ads[t] = ad_t
            # 8 column-tiled matmul pairs: even tile -> partitions 0:64
            # (array col group 0-1), odd tile -> 64:128 (col group 2-3);
            # the two streams use disjoint column groups and overlap
            for p in range(NT // 2):
                nc.tensor.matmul(psda[:],
                                 sel[:, (2 * p) * SELW:(2 * p + 1) * SELW],
                                 ads[2 * p][:],
                                 start=(p == 0), stop=False)
                nc.tensor.matmul(psdb[OUT_FEAT:128, :],
                                 sel[:, (2 * p + 1) * SELW:
                                      (2 * p + 2) * SELW],
                                 ads[2 * p + 1][:],
                                 start=(p == 0), stop=(p == NT // 2 - 1),
                                 tile_position=(0, 64))
            # -S_j correction joins the A bank (group stays open: the
            # fold matmul emitted next iteration closes it)
            nc.tensor.matmul(psda[:], dneg[:], S_bf[:],
                             start=False, stop=False)
            # fold+exp of the PREVIOUS iteration: its PE wait is already
            # satisfied, so the DVE never stalls, and it refreshes DVE's
            # observed PE clock for the elementwise slot reuse
            if pending is not None:
                fold_exp(*pending)
            pending = ((psda, psdb), i)
        fold_exp(*pending)

        # outputs go out on the SW-DGE queues (gpsimd): the HW-DGE queues
        # carried the input loads, and a shared queue would add a second
        # sync-wait command that the DMA pseudo-instruction cannot encode
        nc.gpsimd.dma_start(rows_d[:], rowS[:])
        nc.gpsimd.dma_start(acc_d[:], accS[:])

    if split_waits:
        _split_multiwaits(nc, mybir)
    return nc


def _split_multiwaits(nc, mybir):
    """Walrus on this toolchain encodes at most ONE sync-wait command per
    instruction.  Split any instruction with more waits (in practice only
    the framework's kernel-tail drain) into a chain of single-wait Drain
    carriers on the same engine, inserted immediately before it."""
    n = 0
    for fn in nc.m.functions:
        for bb in fn.blocks:
            new_insts = []
            for inst in bb.instructions:
                si = getattr(inst, "sync_info", None)
                if si is not None and si.on_wait and len(si.on_wait) > 1:
                    waits = list(si.on_wait)
                    for w in waits[:-1]:
                        carrier = mybir.InstDrain(
                            name=f"splitw_{n}", engine=inst.engine,
                            ins=[], outs=[],
                            sync_info=mybir.SyncInfo(on_wait=[w],
                                                     on_update=[]))
                        new_insts.append(carrier)
                        n += 1
                    inst.sync_info = mybir.SyncInfo(
                        on_wait=[waits[-1]], on_update=list(si.on_update))
                new_insts.append(inst)
            if n:
                bb.instructions = new_insts


def _sel_host(value, act_value=None):
    sel = np.zeros((128, NT * SELW), dtype=np.float32)
    for t in range(NT):
        v = value if (act_value is None or t not in ACT_TILES) else act_value
        for g in range(4):
            sel[32 * g:32 * (g + 1), t * SELW + 4 * t + g] = v
    return sel.astype(ml_dtypes.bfloat16)


def _sel2_host():
    s = np.zeros((128, OUT_FEAT), dtype=np.float32)
    s[:OUT_FEAT, :] = np.eye(OUT_FEAT)
    s[OUT_FEAT:, :] = np.eye(OUT_FEAT)
    return s.astype(ml_dtypes.bfloat16)


def _block_order(c):
    """Column blocks for core c; None marks the poison block."""
    if c < 4:
        return [c, c + 1, c + 2, c + 3, c + 4]
    return [c, (c + 1) % 8, (c + 2) % 8, (c + 3) % 8, None]


def _in_maps(x, T):
    bf16 = ml_dtypes.bfloat16
    Tb = np.ascontiguousarray(T.reshape(IN_FEAT, OK)).astype(bf16)
    selb = _sel_host(2.0, act_value=1.0)
    selSb = _sel_host(1.0)
    sel2b = _sel2_host()
    dnegb = (-np.eye(OUT_FEAT, dtype=np.float32)).astype(bf16)
    xT = np.ascontiguousarray(x.T)
    maps = []
    for c in range(N_CORES):
        xTc = np.empty((IN_FEAT, FD), dtype=np.float32)
        for pos, b in enumerate(_block_order(c)):
            if b is None:
                xTc[:, 64 * pos:64 * (pos + 1)] = POISON
            else:
                xTc[:, 64 * pos:64 * (pos + 1)] = xT[:, 64 * b:64 * (b + 1)]
        maps.append({"xT": xTc.astype(bf16), "Tm": Tb, "sel": selb,
                     "selS": selSb, "sel2": sel2b, "dneg": dnegb})
    return maps


def kernel(x, T):
    from concourse import bass_utils

    x = np.asarray(x, dtype=np.float32)
    T = np.asarray(T, dtype=np.float32)

    if "nc" not in _cache:
        _cache["nc"] = _build_nc()
    nc = _cache["nc"]

    res = bass_utils.run_bass_kernel_spmd(
        nc, _in_maps(x, T), core_ids=list(range(N_CORES)))

    mbd = np.zeros((BATCH, OUT_FEAT), dtype=np.float32)
    for c in range(N_CORES):
        rs = np.asarray(res.results[c]["rowS"], dtype=np.float32)  # [o, i]
        mbd[64 * c:64 * (c + 1), :] += rs.T
        acc = np.asarray(res.results[c]["accS"], dtype=np.float32)  # [o, j]
        for pos, b in enumerate(_block_order(c)):
            if pos == 0 or b is None:
                continue  # own diag block is fully in rowsums; poison dropped
            mbd[64 * b:64 * (b + 1), :] += acc[:, 64 * pos:64 * (pos + 1)].T
    mbd -= 1.0
    return np.concatenate([x, mbd], axis=1)



# revision 20
# speedup vs baseline: 1.5140x; 1.5140x over previous
"""MiniBatchDiscrimination kernel, v5: direct |d| + fp8 DoubleRow reduction
+ triangle-trimmed diagonal block + 3-engine elementwise split.

Per core (row block c): columns = 4 partner blocks (c+1..c+4 mod 8) at
positions 0-3 and the OWN (diagonal) block last, cols 256:320.  Per row i
only cols [0, 257+i) are computed (j <= i within the diagonal block); the
j > i half comes from the column sums by symmetry.

  Mt[(o,k), j] bf16 tiles (16), computed on PE, copied out on DVE/ACT/Pool.
  Per i:  A_t = |Mt_t - Mt_t[:, 256+i]| directly:
    tiles 0-9   DVE  tensor_scalar(subtract, abs_max) -> bf16
    tiles 10,11 ACT  Abs(scale=-1, bias=col)          -> fp8e4 (pair 0)
    tiles 12-14 Pool tensor_scalar(subtract, abs_max) -> fp8e4
    tile  15    DVE  tensor_scalar                    -> fp8e4 (pair 2 lo half)
  Reduction over k on PE into one PSUM bank [64, W]: 10 bf16 matmuls
  (64-wide 1.0-selectors) + 3 fp8 DoubleRow matmuls (each contracts a
  PAIR of tiles at 0.5 cycles/row).  No S correction: the diagonal entry
  is exactly 0 because the subtracted scalar is the tile's own column.
  Exp(scale=-1) on ACT reads the PSUM row, packs e tiles (2 rows each),
  accum_out -> rowS.  Column sums per 16-row chunk on PE (descending
  width so the PSUM zero-region is cleared by the first matmul),
  accumulated into accS on Pool.

Host: mbd rows c = rowS_c + accS transposes from cores c-1..c-3 +
own-diagonal accS tail (+1 for odd-row self terms, -2 for the double
counted E_ii and the reference's self-similarity subtraction).
"""

import numpy as np
import ml_dtypes
from contextlib import ExitStack

BATCH, IN_FEAT, OUT_FEAT, KERNEL_DIM = 512, 512, 64, 32
N_CORES = 8
ROWB = BATCH // N_CORES          # 64 rows of i per core
OK = OUT_FEAT * KERNEL_DIM       # 2048 flattened (o,k)
NT = OK // 128                   # 16 partition-tiles of (o,k)
NBLK = 5                         # column blocks per core
FD = NBLK * 64                   # 320
DIAG0 = (NBLK - 1) * 64          # 256: diagonal block start column

DVE_BF = (0, 1, 2, 3, 4, 5, 6, 7, 8, 9)
PAIRS = ((10, 11), (12, 13), (14, 15))
# The DVE/Pool TensorScalar ISA has no (subtract, abs_max): those tiles
# hold relu(d) with weight 2.0 in the selector, corrected by -(S_j - S_i)
# summed over the same tiles (|d| = 2 relu(d) - d).  ACT's Abs is a real
# activation function, so its tiles hold |d| directly with weight 1.0.
PROD = {10: "act", 11: "act", 12: "pool", 13: "pool", 14: "pool",
        15: "dve8"}
RELU_T = DVE_BF + tuple(t for t, e in PROD.items() if e != "act")
CHUNK = 16                       # i's per colsum chunk
SELW = OUT_FEAT

_cache = {}


def _build_nc(split_waits=True):
    import concourse.bass as bass
    import concourse.mybir as mybir
    import concourse.tile as tile

    dt = mybir.dt
    AF = mybir.ActivationFunctionType
    OP = mybir.AluOpType
    DR = mybir.MatmulPerfMode.DoubleRow

    nc = bass.Bass("TRN2", target_bir_lowering=False, debug=False,
                   num_devices=N_CORES)

    xT_d = nc.dram_tensor("xT", [IN_FEAT, FD], dt.bfloat16,
                          kind="ExternalInput")
    T_d = nc.dram_tensor("Tm", [IN_FEAT, OK], dt.bfloat16,
                         kind="ExternalInput")
    selb_d = nc.dram_tensor("selb", [128, len(DVE_BF) * SELW], dt.bfloat16,
                            kind="ExternalInput")
    seldr_d = nc.dram_tensor("seldr", [128, len(PAIRS) * 128], dt.float8e4,
                             kind="ExternalInput")
    sel2_d = nc.dram_tensor("sel2", [128, OUT_FEAT], dt.bfloat16,
                            kind="ExternalInput")
    selS_d = nc.dram_tensor("selS", [128, len(RELU_T) * SELW], dt.bfloat16,
                            kind="ExternalInput")
    dneg_d = nc.dram_tensor("dneg", [OUT_FEAT, OUT_FEAT], dt.bfloat16,
                            kind="ExternalInput")
    rows_d = nc.dram_tensor("rowS", [OUT_FEAT, ROWB], dt.float32,
                            kind="ExternalOutput")
    acc_d = nc.dram_tensor("accS", [OUT_FEAT, FD], dt.float32,
                           kind="ExternalOutput")

    with tile.TileContext(nc) as tc, ExitStack() as ctx:
        const = ctx.enter_context(tc.tile_pool(name="const", bufs=1))
        mtp = ctx.enter_context(tc.tile_pool(name="mt", bufs=NT))
        advp = ctx.enter_context(tc.tile_pool(name="adv", bufs=30))
        pairp = ctx.enter_context(tc.tile_pool(name="pair", bufs=9))
        ep = ctx.enter_context(tc.tile_pool(name="e", bufs=20))
        psda = ctx.enter_context(
            tc.tile_pool(name="psda", bufs=3, space=bass.MemorySpace.PSUM))
        psc = ctx.enter_context(
            tc.tile_pool(name="psc", bufs=2, space=bass.MemorySpace.PSUM))

        Tsb = []
        for kc in range(4):
            t_ = const.tile([128, OK], dt.bfloat16, tag=f"T{kc}")
            nc.sync.dma_start(t_[:], T_d[kc * 128:(kc + 1) * 128, :])
            Tsb.append(t_)
        xTsb = []
        for kc in range(4):
            t_ = const.tile([128, FD], dt.bfloat16, tag=f"x{kc}")
            nc.sync.dma_start(t_[:], xT_d[kc * 128:(kc + 1) * 128, :])
            xTsb.append(t_)
        selb = const.tile([128, len(DVE_BF) * SELW], dt.bfloat16, tag="selb")
        nc.sync.dma_start(selb[:], selb_d[:])
        seldr = const.tile([128, len(PAIRS) * 128], dt.float8e4, tag="seldr")
        nc.sync.dma_start(seldr[:], seldr_d[:])
        sel2 = const.tile([128, OUT_FEAT], dt.bfloat16, tag="sel2")
        nc.sync.dma_start(sel2[:], sel2_d[:])
        selS = const.tile([128, len(RELU_T) * SELW], dt.bfloat16, tag="selS")
        nc.sync.dma_start(selS[:], selS_d[:])
        dneg = const.tile([OUT_FEAT, OUT_FEAT], dt.bfloat16, tag="dneg")
        nc.sync.dma_start(dneg[:], dneg_d[:])
        rowS = const.tile([OUT_FEAT, ROWB], dt.float32, tag="rowS")
        accS = const.tile([OUT_FEAT, FD], dt.float32, tag="accS")
        nc.vector.memset(accS[:], 0.0)
        # fp32 image of the (rounded) bf16 diagonal columns: the subtracted
        # scalar exactly equals the tile value, so D_ii == 0 exactly
        mcol = const.tile([128, NT * ROWB], dt.float32, tag="mcol")

        # ---- M preamble: Mt[(o,k), j] tiles in bf16, copy spread over
        # DVE/ACT/Pool ----
        mts = []
        with tc.tile_pool(name="psm", bufs=2,
                          space=bass.MemorySpace.PSUM) as psm:
            for t in range(NT):
                ps = psm.tile([128, FD], dt.float32, tag="psm", name=f"psm_{t}")
                for kc in range(4):
                    nc.tensor.matmul(ps[:],
                                     Tsb[kc][:, t * 128:(t + 1) * 128],
                                     xTsb[kc][:],
                                     start=(kc == 0), stop=(kc == 3))
                mt_t = mtp.tile([128, FD], dt.bfloat16, tag="mt",
                                name=f"mt_{t}")
                # GPSIMD cannot access PSUM: spread copies over DVE/ACT only
                if t < 8:
                    nc.vector.tensor_copy(mt_t[:], ps[:])
                else:
                    nc.scalar.copy(mt_t[:], ps[:])
                nc.vector.tensor_copy(mcol[:, t * ROWB:(t + 1) * ROWB],
                                      mt_t[:, DIAG0:FD])
                mts.append(mt_t)
            # S over the relu tiles, kept in bf16 so the matmul -S_j and
            # the Exp bias +S_i cancel exactly on the diagonal
            psS = psc.tile([OUT_FEAT, FD], dt.float32, tag="psc",
                           name="psS")
            for m, t in enumerate(RELU_T):
                nc.tensor.matmul(psS[:], selS[:, m * SELW:(m + 1) * SELW],
                                 mts[t][:], start=(m == 0),
                                 stop=(m == len(RELU_T) - 1))
            S_bf = const.tile([OUT_FEAT, FD], dt.bfloat16, tag="S_bf")
            nc.vector.tensor_copy(S_bf[:], psS[:])
            Sneg = const.tile([OUT_FEAT, ROWB], dt.float32, tag="Sneg")
            nc.vector.tensor_scalar(Sneg[:], S_bf[:, DIAG0:FD], -1.0, None,
                                    op0=OP.mult)

        e_lists = [[] for _ in range(ROWB // CHUNK)]
        pend_exp = None

        def emit_exp(i, ps_i):
            W = DIAG0 + i + 1
            ch = i // CHUNK
            if i % 2 == 0:
                e_t = ep.tile([128, FD], dt.bfloat16, tag="e",
                              name=f"e_{i}")
                e_lists[ch].append(e_t)
            half = e_lists[ch][-1][(i % 2) * OUT_FEAT:
                                   (i % 2 + 1) * OUT_FEAT, 0:W]
            nc.scalar.activation(half, ps_i[:, 0:W], AF.Exp,
                                 scale=-1.0, bias=Sneg[:, i:i + 1],
                                 accum_out=rowS[:, i:i + 1])

        def emit_colsum(ch):
            # descending width: first matmul covers the widest range so the
            # PSUM zero-region is fully cleared before narrower accumulates
            pc = psc.tile([OUT_FEAT, FD], dt.float32, tag="psc",
                          name=f"psc_{ch}")
            n = CHUNK // 2
            for m in range(n - 1, -1, -1):
                Wp = DIAG0 + CHUNK * ch + 2 * m + 1
                nc.tensor.matmul(pc[:, 0:Wp], sel2[:],
                                 e_lists[ch][m][:, 0:Wp],
                                 start=(m == n - 1), stop=(m == 0))
            # accumulate on DVE (GPSIMD cannot read PSUM); only the columns
            # this chunk's widest matmul wrote are defined
            Wm = DIAG0 + CHUNK * ch + CHUNK - 1
            nc.vector.tensor_tensor(accS[:, 0:Wm], accS[:, 0:Wm],
                                    pc[:, 0:Wm], op=OP.add)

        for i in range(ROWB):
            W = DIAG0 + i + 1
            ps_i = psda.tile([OUT_FEAT, FD], dt.float32, tag="psda",
                             name=f"psda_{i}")
            # ---- elementwise |d| tiles ----
            ads = {}
            pbs = []
            for pi, (ta, tb) in enumerate(PAIRS):
                pb = pairp.tile([128, 2 * FD], dt.float8e4, tag="pb",
                                name=f"pb_{i}_{pi}")
                pbs.append(pb)
                ads[ta] = pb[:, 0:W]
                ads[tb] = pb[:, FD:FD + W]
            # DVE fp8 half first so its consumer (last DR matmul) is ready
            for t, eng in PROD.items():
                if eng != "dve8":
                    continue
                sc = mcol[:, t * ROWB + i:t * ROWB + i + 1]
                nc.vector.tensor_scalar(ads[t], mts[t][:, 0:W], sc, 0.0,
                                        op0=OP.subtract, op1=OP.max)
            for t in DVE_BF:
                sc = mcol[:, t * ROWB + i:t * ROWB + i + 1]
                ad_t = advp.tile([128, FD], dt.bfloat16, tag="adv",
                                 name=f"ad_{i}_{t}")
                nc.vector.tensor_scalar(ad_t[:, 0:W], mts[t][:, 0:W], sc,
                                        0.0, op0=OP.subtract, op1=OP.max)
                ads[t] = ad_t[:, 0:W]
            for t, eng in PROD.items():
                sc = mcol[:, t * ROWB + i:t * ROWB + i + 1]
                if eng == "act":
                    nc.scalar.activation(ads[t], mts[t][:, 0:W], AF.Abs,
                                         bias=sc, scale=-1.0)
                elif eng == "pool":
                    nc.gpsimd.tensor_scalar(ads[t], mts[t][:, 0:W], sc, 0.0,
                                            op0=OP.subtract, op1=OP.max)
            # ---- reduction matmuls into one PSUM bank ----
            for m, t in enumerate(DVE_BF):
                nc.tensor.matmul(ps_i[:, 0:W],
                                 selb[:, m * SELW:(m + 1) * SELW],
                                 ads[t], start=(m == 0), stop=False)
            for pi in range(len(PAIRS)):
                nc.tensor.matmul(
                    ps_i[:, 0:W],
                    seldr[:, pi * 128:(pi + 1) * 128].rearrange(
                        "p (two m) -> p two m", two=2),
                    pbs[pi][:, 0:2 * FD].rearrange(
                        "p (two n) -> p two n", two=2)[:, :, 0:W],
                    start=False, stop=False,
                    perf_mode=DR)
            # -S_j correction for the relu (pool) tiles
            nc.tensor.matmul(ps_i[:, 0:W], dneg[:], S_bf[:, 0:W],
                             start=False, stop=True)
            # ---- software-pipelined Exp / colsum ----
            if pend_exp is not None:
                emit_exp(*pend_exp)
            pend_exp = (i, ps_i)
            if i % CHUNK == 1 and i > CHUNK:
                emit_colsum(i // CHUNK - 1)
        emit_exp(*pend_exp)
        emit_colsum(ROWB // CHUNK - 1)

        nc.gpsimd.dma_start(rows_d[:], rowS[:])
        nc.gpsimd.dma_start(acc_d[:], accS[:])

    if split_waits:
        _split_multiwaits(nc, mybir)
    return nc


def _split_multiwaits(nc, mybir):
    """Walrus encodes at most ONE sync-wait command per instruction. Split
    any instruction with more into a chain of single-wait Drain carriers on
    the same engine, inserted immediately before it."""
    n = 0
    for fn in nc.m.functions:
        for bb in fn.blocks:
            new_insts = []
            for inst in bb.instructions:
                si = getattr(inst, "sync_info", None)
                if si is not None and si.on_wait and len(si.on_wait) > 1:
                    waits = list(si.on_wait)
                    for w in waits[:-1]:
                        carrier = mybir.InstDrain(
                            name=f"splitw_{n}", engine=inst.engine,
                            ins=[], outs=[],
                            sync_info=mybir.SyncInfo(on_wait=[w],
                                                     on_update=[]))
                        new_insts.append(carrier)
                        n += 1
                    inst.sync_info = mybir.SyncInfo(
                        on_wait=[waits[-1]], on_update=list(si.on_update))
                new_insts.append(inst)
            if n:
                bb.instructions = new_insts


def _selb_host():
    sel = np.zeros((128, len(DVE_BF) * SELW), dtype=np.float32)
    for m, t in enumerate(DVE_BF):
        for g in range(4):
            sel[32 * g:32 * (g + 1), m * SELW + 4 * t + g] = 2.0
    return sel.astype(ml_dtypes.bfloat16)


def _seldr_host():
    sel = np.zeros((128, len(PAIRS) * 128), dtype=np.float32)
    for pi, pair in enumerate(PAIRS):
        for h, t in enumerate(pair):
            v = 1.0 if PROD[t] == "act" else 2.0
            for g in range(4):
                sel[32 * g:32 * (g + 1),
                    pi * 128 + h * SELW + 4 * t + g] = v
    return sel.astype(ml_dtypes.float8_e4m3)


def _selS_host():
    sel = np.zeros((128, len(RELU_T) * SELW), dtype=np.float32)
    for m, t in enumerate(RELU_T):
        for g in range(4):
            sel[32 * g:32 * (g + 1), m * SELW + 4 * t + g] = 1.0
    return sel.astype(ml_dtypes.bfloat16)


def _sel2_host():
    s = np.zeros((128, OUT_FEAT), dtype=np.float32)
    s[:OUT_FEAT, :] = np.eye(OUT_FEAT)
    s[OUT_FEAT:, :] = np.eye(OUT_FEAT)
    return s.astype(ml_dtypes.bfloat16)


def _block_order(c):
    """Column blocks for core c: partners c+1..c+4 (mod 8), own block last."""
    return [(c + 1 + s) % 8 for s in range(4)] + [c]


def _in_maps(x, T):
    bf16 = ml_dtypes.bfloat16
    Tb = np.ascontiguousarray(T.reshape(IN_FEAT, OK)).astype(bf16)
    selb = _selb_host()
    seldr = _seldr_host()
    sel2b = _sel2_host()
    selSb = _selS_host()
    dnegb = (-np.eye(OUT_FEAT, dtype=np.float32)).astype(bf16)
    xT = np.ascontiguousarray(x.T)
    maps = []
    for c in range(N_CORES):
        xTc = np.empty((IN_FEAT, FD), dtype=np.float32)
        for pos, b in enumerate(_block_order(c)):
            xTc[:, 64 * pos:64 * (pos + 1)] = xT[:, 64 * b:64 * (b + 1)]
        maps.append({"xT": xTc.astype(bf16), "Tm": Tb, "selb": selb,
                     "seldr": seldr, "sel2": sel2b, "selS": selSb,
                     "dneg": dnegb})
    return maps


def _assemble(x, results):
    """results: list of 8 dicts with 'rowS' [o, i] and 'accS' [o, 320]."""
    mbd = np.zeros((BATCH, OUT_FEAT), dtype=np.float32)
    for c in range(N_CORES):
        rs = np.asarray(results[c]["rowS"], dtype=np.float32)
        acc = np.asarray(results[c]["accS"], dtype=np.float32).copy()
        # odd rows' diagonal self term (E_ii = 1) is missing from the
        # pair-packed column sums
        acc[:, DIAG0 + 1::2] += 1.0
        # own rows: row sums (j <= i) + diagonal transpose tail (j > i,
        # = accS diag col i minus the double-counted E_ii)
        mbd[64 * c:64 * (c + 1), :] += rs.T + acc[:, DIAG0:].T - 1.0
        # partner transposes: positions 0..2 (blocks c+1..c+3); position 3
        # (c+4) is the duplicated pair, covered by that core's own rowS
        for s in range(3):
            b = (c + 1 + s) % 8
            mbd[64 * b:64 * (b + 1), :] += acc[:, 64 * s:64 * (s + 1)].T
    mbd -= 1.0  # reference subtracts the self-similarity exp(0)=1
    return np.concatenate([np.asarray(x, np.float32), mbd], axis=1)


def kernel(x, T):
    from concourse import bass_utils

    x = np.asarray(x, dtype=np.float32)
    T = np.asarray(T, dtype=np.float32)

    if "nc" not in _cache:
        _cache["nc"] = _build_nc()
    nc = _cache["nc"]

    res = bass_utils.run_bass_kernel_spmd(
        nc, _in_maps(x, T), core_ids=list(range(N_CORES)))
    return _assemble(x, res.results)


# revision 23
# speedup vs baseline: 1.6328x; 1.0785x over previous
"""MiniBatchDiscrimination kernel, v5.1: fp8 DoubleRow everywhere it pays
+ triangle-trimmed diagonal block + 3-engine elementwise split.

Per core (row block c): columns = 4 partner blocks (c+1..c+4 mod 8) at
positions 0-3 and the OWN (diagonal) block last, cols 256:320.  Per row i
only cols [0, 257+i) are computed (j <= i within the diagonal block); the
j > i half comes from the column sums by symmetry (4 block pairs are
computed twice, once per orientation, so every core carries 5 real
blocks and no poison).

  M = x @ T on PE as fp8 DoubleRow matmuls: inputs are host-quantized
  x*ax, T*aT with ax, aT POWERS OF TWO, so the compensation beta =
  2^12/(ax aT) is exact in bf16 and fp8 and is folded into the selector
  weights at zero cost.  Mt'' tiles (bf16) = psum * 2^-12 (fixed scale).
  Per i:  d_t = Mt_t - Mt_t[:, 256+i] per tile:
    tiles 0-9   DVE  relu(d) -> bf16     (weight 2 beta, bf16 matmuls)
    tiles 10,11 ACT  |d| via Abs -> fp8  (weight beta, DR pair 0)
    tiles 12-14 Pool relu(d) -> fp8     (weight 2 beta, DR pairs 1-2)
    tile  15    DVE  relu(d) -> fp8     (weight 2 beta, DR pair 2)
  (the DVE/Pool TensorScalar ISA has no (subtract, abs_max), hence relu
  with |d| = 2 relu(d) - d and the S correction below; ACT's Abs is a
  real activation function.)
  Reduction on PE into one PSUM bank [64, W]: 10 bf16 matmuls + 3 fp8
  DoubleRow pair matmuls + 1 DR pair (-beta * S8, 0-weight dummy) that
  applies the -S_j correction.  S8 = fp8(sum_k Mt'' over relu tiles).
  The Exp bias +S_i is -nbeta*S8[:, 256+i] computed from the SAME fp8
  values, so D_ii == 0 exactly and exp(-0) = 1 lands in rowS/accS.
  Exp on ACT packs e tiles (2 rows each) with accum_out -> rowS.
  Column sums per 16-row chunk on PE: a zero-weight full-width matmul
  opens the PSUM accumulation group, then one pair matmul right after
  each odd row's Exp (ascending widths), accumulated into accS on DVE.

Host: mbd rows c = rowS_c + accS transposes from cores c-1..c-3 +
own-diagonal accS tail (+1 for odd rows' self term missing from the
pair-packed column sums, -1 for the double-counted E_ii, -1 for the
reference's self-similarity subtraction).
"""

import math

import numpy as np
import ml_dtypes
from contextlib import ExitStack

BATCH, IN_FEAT, OUT_FEAT, KERNEL_DIM = 512, 512, 64, 32
N_CORES = 8
ROWB = BATCH // N_CORES          # 64 rows of i per core
OK = OUT_FEAT * KERNEL_DIM       # 2048 flattened (o,k)
NT = OK // 128                   # 16 partition-tiles of (o,k)
NBLK = 5                         # column blocks per core
FD = NBLK * 64                   # 320
DIAG0 = (NBLK - 1) * 64          # 256: diagonal block start column

DVE_BF = (0, 1, 2, 3, 4, 5, 6, 7, 8, 9)
PAIRS = ((10, 11), (12, 13), (14, 15))
PROD = {10: "act", 11: "act", 12: "pool", 13: "pool", 14: "pool",
        15: "dve8"}
RELU_T = DVE_BF + tuple(t for t, e in PROD.items() if e != "act")
CHUNK = 16                       # i's per colsum chunk
SELW = OUT_FEAT

_cache = {}


def _build_nc(split_waits=True):
    import concourse.bass as bass
    import concourse.mybir as mybir
    import concourse.tile as tile

    dt = mybir.dt
    AF = mybir.ActivationFunctionType
    OP = mybir.AluOpType
    DR = mybir.MatmulPerfMode.DoubleRow

    nc = bass.Bass("TRN2", target_bir_lowering=False, debug=False,
                   num_devices=N_CORES)

    # fp8 inputs for the DoubleRow M matmuls: pack q holds infeat chunks
    # (2q, 2q+1) interleaved in the free dim (the DR pair dimension)
    T_d = nc.dram_tensor("Tm", [2 * 128, 2 * OK], dt.float8e4,
                         kind="ExternalInput")
    xT_d = nc.dram_tensor("xT", [2 * 128, 2 * FD], dt.float8e4,
                          kind="ExternalInput")
    selb_d = nc.dram_tensor("selb", [128, len(DVE_BF) * SELW], dt.bfloat16,
                            kind="ExternalInput")
    seldr_d = nc.dram_tensor("seldr", [128, (len(PAIRS) + 1) * 128],
                             dt.float8e4, kind="ExternalInput")
    sel2_d = nc.dram_tensor("sel2", [128, OUT_FEAT], dt.bfloat16,
                            kind="ExternalInput")
    selS_d = nc.dram_tensor("selS", [128, len(RELU_T) * SELW], dt.bfloat16,
                            kind="ExternalInput")
    nbeta_d = nc.dram_tensor("nbeta", [OUT_FEAT, 1], dt.float32,
                             kind="ExternalInput")
    rows_d = nc.dram_tensor("rowS", [OUT_FEAT, ROWB], dt.float32,
                            kind="ExternalOutput")
    acc_d = nc.dram_tensor("accS", [OUT_FEAT, FD], dt.float32,
                           kind="ExternalOutput")

    with tile.TileContext(nc) as tc, ExitStack() as ctx:
        const = ctx.enter_context(tc.tile_pool(name="const", bufs=1))
        mtp = ctx.enter_context(tc.tile_pool(name="mt", bufs=NT))
        advp = ctx.enter_context(tc.tile_pool(name="adv", bufs=30))
        pairp = ctx.enter_context(tc.tile_pool(name="pair", bufs=9))
        ep = ctx.enter_context(tc.tile_pool(name="e", bufs=20))
        psda = ctx.enter_context(
            tc.tile_pool(name="psda", bufs=3, space=bass.MemorySpace.PSUM))
        psc = ctx.enter_context(
            tc.tile_pool(name="psc", bufs=2, space=bass.MemorySpace.PSUM))

        # ---- input DMAs: first-needed first, spread over the SP/ACT/DVE
        # HWDGE queues so the first M matmul can start after ~1.5us ----
        Tsb, xsb = [], []
        for q in range(2):
            t_ = const.tile([128, 2 * OK], dt.float8e4, tag=f"T{q}")
            Tsb.append(t_)
            x_ = const.tile([128, 2 * FD], dt.float8e4, tag=f"x{q}")
            xsb.append(x_)
        nc.sync.dma_start(Tsb[0][:], T_d[0:128, :])
        nc.sync.dma_start(xsb[0][:], xT_d[0:128, :])
        nc.scalar.dma_start(Tsb[1][:], T_d[128:256, :])
        nc.scalar.dma_start(xsb[1][:], xT_d[128:256, :])
        selb = const.tile([128, len(DVE_BF) * SELW], dt.bfloat16, tag="selb")
        nc.sync.dma_start(selb[:], selb_d[:])
        seldr = const.tile([128, (len(PAIRS) + 1) * 128], dt.float8e4,
                           tag="seldr")
        nc.sync.dma_start(seldr[:], seldr_d[:])
        sel2 = const.tile([128, OUT_FEAT], dt.bfloat16, tag="sel2")
        nc.sync.dma_start(sel2[:], sel2_d[:])
        selS = const.tile([128, len(RELU_T) * SELW], dt.bfloat16, tag="selS")
        nc.sync.dma_start(selS[:], selS_d[:])
        nbeta = const.tile([OUT_FEAT, 1], dt.float32, tag="nbeta")
        nc.sync.dma_start(nbeta[:], nbeta_d[:])

        rowS = const.tile([OUT_FEAT, ROWB], dt.float32, tag="rowS")
        accS = const.tile([OUT_FEAT, FD], dt.float32, tag="accS")
        nc.vector.memset(accS[:], 0.0)
        # fp32 image of the (rounded) bf16 diagonal columns: the subtracted
        # scalar exactly equals the tile value, so d_ii == 0 exactly
        mcol = const.tile([128, NT * ROWB], dt.float32, tag="mcol")
        # zero weights: opens each colsum PSUM group at full width
        zw = const.tile([128, SELW], dt.bfloat16, tag="zw")
        nc.vector.memset(zw[:], 0.0)

        # ---- M preamble: Mt''[(o,k), j] bf16 tiles via DR matmuls ----
        mts = []
        with tc.tile_pool(name="psm", bufs=2,
                          space=bass.MemorySpace.PSUM) as psm:
            for t in range(NT):
                ps = psm.tile([128, FD], dt.float32, tag="psm",
                              name=f"psm_{t}")
                for q in range(2):
                    nc.tensor.matmul(
                        ps[:],
                        Tsb[q][:].rearrange("p (two m) -> p two m",
                                            two=2)[:, :,
                                                   t * 128:(t + 1) * 128],
                        xsb[q][:].rearrange("p (two n) -> p two n", two=2),
                        start=(q == 0), stop=(q == 1), perf_mode=DR)
                mt_t = mtp.tile([128, FD], dt.bfloat16, tag="mt",
                                name=f"mt_{t}")
                # descale by the FIXED 2^-12 (the data-dependent remainder
                # beta lives in the selector weights)
                if t < 8:
                    nc.vector.tensor_scalar(mt_t[:], ps[:], 2.0 ** -12,
                                            None, op0=OP.mult)
                else:
                    nc.scalar.activation(mt_t[:], ps[:], AF.Copy,
                                         scale=2.0 ** -12)
                nc.scalar.activation(mcol[:, t * ROWB:(t + 1) * ROWB],
                                     mt_t[:, DIAG0:FD], AF.Copy)
                mts.append(mt_t)
            # S'' over the relu tiles; kept in fp8 so the -beta*S8 matmul
            # and the Exp bias +beta*S8_i cancel exactly on the diagonal
            psS = psc.tile([OUT_FEAT, FD], dt.float32, tag="psc",
                           name="psS")
            for m, t in enumerate(RELU_T):
                nc.tensor.matmul(psS[:], selS[:, m * SELW:(m + 1) * SELW],
                                 mts[t][:], start=(m == 0),
                                 stop=(m == len(RELU_T) - 1))
            S8 = const.tile([OUT_FEAT, 2 * FD], dt.float8e4, tag="S8")
            nc.scalar.activation(S8[:, 0:FD], psS[:], AF.Copy)
            nc.scalar.activation(S8[:, FD:2 * FD], psS[:], AF.Copy)
            Sneg = const.tile([OUT_FEAT, ROWB], dt.float32, tag="Sneg")
            nc.vector.tensor_scalar(Sneg[:], S8[:, DIAG0:FD], nbeta[:],
                                    None, op0=OP.mult)

        e_lists = [[] for _ in range(ROWB // CHUNK)]
        pend_exp = None
        pc_cur = [None]

        def emit_exp(i, ps_i):
            W = DIAG0 + i + 1
            ch = i // CHUNK
            if i % 2 == 0:
                e_t = ep.tile([128, FD], dt.bfloat16, tag="e",
                              name=f"e_{i}")
                e_lists[ch].append(e_t)
            half = e_lists[ch][-1][(i % 2) * OUT_FEAT:
                                   (i % 2 + 1) * OUT_FEAT, 0:W]
            nc.scalar.activation(half, ps_i[:, 0:W], AF.Exp,
                                 scale=-1.0, bias=Sneg[:, i:i + 1],
                                 accum_out=rowS[:, i:i + 1])
            if i % 2 == 1:
                # colsum pair matmul for rows (i-1, i), ascending widths;
                # a zero-weight full-width matmul opened the group so the
                # PSUM zero region is already cleared
                m = (i % CHUNK) // 2
                if m == 0:
                    pc_cur[0] = psc.tile([OUT_FEAT, FD], dt.float32,
                                         tag="psc", name=f"psc_{ch}")
                    nc.tensor.matmul(pc_cur[0][:], zw[:, 0:SELW],
                                     mts[0][:], start=True, stop=False)
                Wp = DIAG0 + CHUNK * ch + 2 * m + 1
                nc.tensor.matmul(pc_cur[0][:, 0:Wp], sel2[:],
                                 e_lists[ch][m][:, 0:Wp],
                                 start=False, stop=(m == CHUNK // 2 - 1))
                if m == CHUNK // 2 - 1:
                    Wm = DIAG0 + CHUNK * ch + CHUNK - 1
                    nc.vector.tensor_tensor(accS[:, 0:Wm], accS[:, 0:Wm],
                                            pc_cur[0][:, 0:Wm], op=OP.add)

        for i in range(ROWB):
            W = DIAG0 + i + 1
            ps_i = psda.tile([OUT_FEAT, FD], dt.float32, tag="psda",
                             name=f"psda_{i}")
            # ---- elementwise tiles ----
            ads = {}
            pbs = []
            for pi, (ta, tb) in enumerate(PAIRS):
                pb = pairp.tile([128, 2 * FD], dt.float8e4, tag="pb",
                                name=f"pb_{i}_{pi}")
                pbs.append(pb)
                ads[ta] = pb[:, 0:W]
                ads[tb] = pb[:, FD:FD + W]
            # DVE fp8 half first so its consumer (last DR matmul) is ready
            for t, eng in PROD.items():
                if eng != "dve8":
                    continue
                sc = mcol[:, t * ROWB + i:t * ROWB + i + 1]
                nc.vector.tensor_scalar(ads[t], mts[t][:, 0:W], sc, 0.0,
                                        op0=OP.subtract, op1=OP.max)
            for t in DVE_BF:
                sc = mcol[:, t * ROWB + i:t * ROWB + i + 1]
                ad_t = advp.tile([128, FD], dt.bfloat16, tag="adv",
                                 name=f"ad_{i}_{t}")
                nc.vector.tensor_scalar(ad_t[:, 0:W], mts[t][:, 0:W], sc,
                                        0.0, op0=OP.subtract, op1=OP.max)
                ads[t] = ad_t[:, 0:W]
            for t, eng in PROD.items():
                sc = mcol[:, t * ROWB + i:t * ROWB + i + 1]
                if eng == "act":
                    nc.scalar.activation(ads[t], mts[t][:, 0:W], AF.Abs,
                                         bias=sc, scale=-1.0)
                elif eng == "pool":
                    nc.gpsimd.tensor_scalar(ads[t], mts[t][:, 0:W], sc, 0.0,
                                            op0=OP.subtract, op1=OP.max)
            # ---- reduction matmuls into one PSUM bank ----
            for m, t in enumerate(DVE_BF):
                nc.tensor.matmul(ps_i[:, 0:W],
                                 selb[:, m * SELW:(m + 1) * SELW],
                                 ads[t], start=(m == 0), stop=False)
            for pi in range(len(PAIRS)):
                nc.tensor.matmul(
                    ps_i[:, 0:W],
                    seldr[:, pi * 128:(pi + 1) * 128].rearrange(
                        "p (two m) -> p two m", two=2),
                    pbs[pi][:, 0:2 * FD].rearrange(
                        "p (two n) -> p two n", two=2)[:, :, 0:W],
                    start=False, stop=False,
                    perf_mode=DR)
            # -S_j correction: DR pair (-beta * S8, zero-weight dummy)
            npi = len(PAIRS)
            nc.tensor.matmul(
                ps_i[:, 0:W],
                seldr[0:OUT_FEAT,
                      npi * 128:(npi + 1) * 128].rearrange(
                    "p (two m) -> p two m", two=2),
                S8[:, 0:2 * FD].rearrange(
                    "p (two n) -> p two n", two=2)[:, :, 0:W],
                start=False, stop=True, perf_mode=DR)
            # ---- software-pipelined Exp (+ inline colsum pairs) ----
            if pend_exp is not None:
                emit_exp(*pend_exp)
            pend_exp = (i, ps_i)
        emit_exp(*pend_exp)

        nc.gpsimd.dma_start(rows_d[:], rowS[:])
        nc.gpsimd.dma_start(acc_d[:], accS[:])

    if split_waits:
        _split_multiwaits(nc, mybir)
    return nc


def _split_multiwaits(nc, mybir):
    """Walrus encodes at most ONE sync-wait command per instruction. Split
    any instruction with more into a chain of single-wait Drain carriers on
    the same engine, inserted immediately before it."""
    n = 0
    for fn in nc.m.functions:
        for bb in fn.blocks:
            new_insts = []
            for inst in bb.instructions:
                si = getattr(inst, "sync_info", None)
                if si is not None and si.on_wait and len(si.on_wait) > 1:
                    waits = list(si.on_wait)
                    for w in waits[:-1]:
                        carrier = mybir.InstDrain(
                            name=f"splitw_{n}", engine=inst.engine,
                            ins=[], outs=[],
                            sync_info=mybir.SyncInfo(on_wait=[w],
                                                     on_update=[]))
                        new_insts.append(carrier)
                        n += 1
                    inst.sync_info = mybir.SyncInfo(
                        on_wait=[waits[-1]], on_update=list(si.on_update))
                new_insts.append(inst)
            if n:
                bb.instructions = new_insts


def _pow2_scale(m, target=200.0):
    """Largest power of two a with m * a <= target."""
    if not np.isfinite(m) or m <= 0:
        return 1.0
    return 2.0 ** math.floor(math.log2(target / m))


def _selb_host(beta):
    sel = np.zeros((128, len(DVE_BF) * SELW), dtype=np.float32)
    for m, t in enumerate(DVE_BF):
        for g in range(4):
            sel[32 * g:32 * (g + 1), m * SELW + 4 * t + g] = 2.0 * beta
    return sel.astype(ml_dtypes.bfloat16)


def _seldr_host(beta):
    sel = np.zeros((128, (len(PAIRS) + 1) * 128), dtype=np.float32)
    for pi, pair in enumerate(PAIRS):
        for h, t in enumerate(pair):
            v = beta if PROD[t] == "act" else 2.0 * beta
            for g in range(4):
                sel[32 * g:32 * (g + 1),
                    pi * 128 + h * SELW + 4 * t + g] = v
    # pair 3: half0 = -beta * I64 (the S correction), half1 = 0
    npi = len(PAIRS)
    sel[0:OUT_FEAT, npi * 128:npi * 128 + OUT_FEAT] = \
        -beta * np.eye(OUT_FEAT, dtype=np.float32)
    return sel.astype(ml_dtypes.float8_e4m3)


def _selS_host():
    sel = np.zeros((128, len(RELU_T) * SELW), dtype=np.float32)
    for m, t in enumerate(RELU_T):
        for g in range(4):
            sel[32 * g:32 * (g + 1), m * SELW + 4 * t + g] = 1.0
    return sel.astype(ml_dtypes.bfloat16)


def _sel2_host():
    s = np.zeros((128, OUT_FEAT), dtype=np.float32)
    s[:OUT_FEAT, :] = np.eye(OUT_FEAT)
    s[OUT_FEAT:, :] = np.eye(OUT_FEAT)
    return s.astype(ml_dtypes.bfloat16)


def _block_order(c):
    """Column blocks for core c: partners c+1..c+4 (mod 8), own block last."""
    return [(c + 1 + s) % 8 for s in range(4)] + [c]


def _pack_pairs(a):
    """[512, n] -> [256, 2n]: infeat chunks (2q, 2q+1) interleaved in the
    free dim (the DoubleRow pair dimension)."""
    n = a.shape[1]
    out = np.empty((256, 2 * n), dtype=a.dtype)
    for q in range(2):
        out[q * 128:(q + 1) * 128, 0:n] = a[(2 * q) * 128:(2 * q + 1) * 128]
        out[q * 128:(q + 1) * 128, n:2 * n] = \
            a[(2 * q + 1) * 128:(2 * q + 2) * 128]
    return out


def _in_maps(x, T):
    f8 = ml_dtypes.float8_e4m3
    ax = _pow2_scale(float(np.abs(x).max()))
    aT = _pow2_scale(float(np.abs(T).max()))
    beta = 2.0 ** 12 / (ax * aT)
    T2 = T.reshape(IN_FEAT, OK)
    Tb = _pack_pairs((T2 * aT).astype(f8))
    selb = _selb_host(beta)
    seldr = _seldr_host(beta)
    sel2b = _sel2_host()
    selSb = _selS_host()
    nbeta = np.full((OUT_FEAT, 1), -beta, dtype=np.float32)
    xT = np.ascontiguousarray(x.T) * ax
    maps = []
    for c in range(N_CORES):
        xTc = np.empty((IN_FEAT, FD), dtype=np.float32)
        for pos, b in enumerate(_block_order(c)):
            xTc[:, 64 * pos:64 * (pos + 1)] = xT[:, 64 * b:64 * (b + 1)]
        maps.append({"xT": _pack_pairs(xTc.astype(f8)), "Tm": Tb,
                     "selb": selb, "seldr": seldr, "sel2": sel2b,
                     "selS": selSb, "nbeta": nbeta})
    return maps


def _assemble(x, results):
    """results: list of 8 dicts with 'rowS' [o, i] and 'accS' [o, 320]."""
    mbd = np.zeros((BATCH, OUT_FEAT), dtype=np.float32)
    for c in range(N_CORES):
        rs = np.asarray(results[c]["rowS"], dtype=np.float32)
        acc = np.asarray(results[c]["accS"], dtype=np.float32).copy()
        # odd rows' diagonal self term (E_ii = 1) is missing from the
        # pair-packed column sums
        acc[:, DIAG0 + 1::2] += 1.0
        # own rows: row sums (j <= i) + diagonal transpose tail (j > i,
        # = accS diag col i minus the double-counted E_ii)
        mbd[64 * c:64 * (c + 1), :] += rs.T + acc[:, DIAG0:].T - 1.0
        # partner transposes: positions 0..2 (blocks c+1..c+3); position 3
        # (c+4) is the duplicated pair, covered by that core's own rowS
        for s in range(3):
            b = (c + 1 + s) % 8
            mbd[64 * b:64 * (b + 1), :] += acc[:, 64 * s:64 * (s + 1)].T
    mbd -= 1.0  # reference subtracts the self-similarity exp(0)=1
    return np.concatenate([np.asarray(x, np.float32), mbd], axis=1)


def kernel(x, T):
    from concourse import bass_utils

    x = np.asarray(x, dtype=np.float32)
    T = np.asarray(T, dtype=np.float32)

    if "nc" not in _cache:
        _cache["nc"] = _build_nc()
    nc = _cache["nc"]

    res = bass_utils.run_bass_kernel_spmd(
        nc, _in_maps(x, T), core_ids=list(range(N_CORES)))
    return _assemble(x, res.results)
